# revision 6
# baseline (speedup 1.0000x reference)
"""YOLOv1 loss kernel for Trainium2, data-parallel over 8 NeuronCores.

Full inputs: pred [16384,30,7,7] f32, labels [16384,30,7,7] f32 -> scalar f32.

Sharding: batch 16384 -> 8 cores x 2048 rows. Per core the kernel streams
pred [2048,1470] and a host-packed labels tensor [2048,1225] (channels 0-4
and 10-29; channels 5-9 are exact duplicates / unused in the reference),
computes the per-cell loss fully on-chip and reduces to [128, NCHUNK]
partial sums. Host sums the 8*128*NCHUNK partials and divides by B.

Math notes (all equivalent to the reference up to f32 rounding):
  - The grid offsets m,n cancel inside the IOU (equal shift of both boxes),
    and scaling all coords by 7 cancels in inter/union, so
    lo = x - 3.5w, hi = x + 3.5w, inter_raw = 49*inter,
    den = 49*(a1+ag) - inter_raw, iou = inter_raw/den.
  - a = w*h equals the reference's (x2-x1)*(y2-y1).
  - den >= 49*ag - ulp > 0 always (labels w,h >= 0.05), so the where() guard
    in the reference is unnecessary: inter==0 already gives iou = 0/den = 0.
  - obj = labels[:,4] exactly (conf is exactly 0/1).
  - inner = U2 + resp*(U1-U2) + cls with U1 = 5c1 + o1 + 0.5o2,
    U2 = 5c2 + o2 + 0.5o1; cell = obj*(inner - sph) + sph,
    sph = 0.5*(p4^2+p9^2).
"""

import numpy as np

import concourse.bass as bass
import concourse.mybir as mybir
import concourse.tile as tile
from concourse import bacc
from concourse.bass_utils import run_bass_kernel_spmd

F32 = mybir.dt.float32
OP = mybir.AluOpType
AF = mybir.ActivationFunctionType

NCORES = 8
B = 16384
BLOC = B // NCORES        # 2048 rows per core
P = 128                   # SBUF partitions
K = 4                     # 128-row blocks processed per chunk
NBLK = BLOC // P          # 16
NCHUNK = NBLK // K        # 4
PREDW = 30 * 49           # 1470
LABW = 25 * 49            # 1225 (channels 0-4 + 10-29)
W = K * 49                # 196: one channel across the K blocks

SQ5 = float(np.float32(np.sqrt(5.0)))
ISQ2 = float(np.float32(np.sqrt(0.5)))


def _body(tc, pred_ap, labs_ap, out_ap):
    nc = tc.nc
    nv = nc.vector
    na = nc.scalar
    ng = nc.gpsimd

    # DRAM views: row index = chunk*K*P + blk*P + p ; DMA iterates [p, k, f].
    pred_r = pred_ap.rearrange("(c k p) f -> c p k f", c=NCHUNK, k=K, p=P)
    labs_r = labs_ap.rearrange("(c k p) f -> c p k f", c=NCHUNK, k=K, p=P)

    import contextlib
    ctx = contextlib.ExitStack()
    with ctx:
        inp = ctx.enter_context(tc.tile_pool(name="inp", bufs=2))
        med = ctx.enter_context(tc.tile_pool(name="med", bufs=1))
        sml = ctx.enter_context(tc.tile_pool(name="sml", bufs=2))
        opool = ctx.enter_context(tc.tile_pool(name="opool", bufs=1))

        acc = opool.tile([P, NCHUNK], F32)

        for c in range(NCHUNK):
            PT = inp.tile([P, K * PREDW], F32, tag="PT")
            LT = inp.tile([P, K * LABW], F32, tag="LT")
            nc.sync.dma_start(
                PT[:].rearrange("p (k f) -> p k f", k=K), pred_r[c])
            nc.sync.dma_start(
                LT[:].rearrange("p (k f) -> p k f", k=K), labs_r[c])

            # pred grouped as 6 groups of 5 channels (245 cols each)
            PTg = PT[:].rearrange("p (k g f) -> p k g f", k=K, g=6)
            LT3 = LT[:].rearrange("p (k f) -> p k f", k=K)

            p_cls = PT[:].rearrange("p (k f) -> p k f", k=K)[:, :, 490:1470]
            l_xy = LT3[:, :, 0:98]
            l_wh = LT3[:, :, 98:196]
            l_w = LT3[:, :, 98:147]
            l_h = LT3[:, :, 147:196]
            l_obj = LT3[:, :, 196:245]
            l_cls = LT3[:, :, 245:1225]

            def t2(name, cols, pool=med):
                # tile with 3D view [p, K, cols]
                t = pool.tile([P, K * cols], F32, tag=name)
                return t, t[:].rearrange("p (k f) -> p k f", k=K)

            def t4(name, x, cols, pool=med):
                # tile holding x interleaved blocks per k:
                #   4D view [p, K, x, cols] and flat 3D view [p, K*x, cols]
                t = pool.tile([P, K * x * cols], F32, tag=name)
                v4 = t[:].rearrange("p (k x f) -> p k x f", k=K, x=x)
                v3 = t[:].rearrange("p (q f) -> p q f", q=K * x)
                return t, v4, v3

            # ---- boxes: lo = xy - 3.5*wh, hi = xy + 3.5*wh (coords x7) ----
            _, lo_p, _ = t4("lo_p", 2, 98)
            _, hi_p, _ = t4("hi_p", 2, 98)
            _, lo_g = t2("lo_g", 98)
            _, hi_g = t2("hi_g", 98)
            for x in range(2):
                wh = PTg[:, :, x, 98:196]
                xy = PTg[:, :, x, 0:98]
                nv.scalar_tensor_tensor(lo_p[:, :, x, :], wh, -3.5, xy,
                                        OP.mult, OP.add)
                nv.scalar_tensor_tensor(hi_p[:, :, x, :], wh, 3.5, xy,
                                        OP.mult, OP.add)
            nv.scalar_tensor_tensor(lo_g, l_wh, -3.5, l_xy, OP.mult, OP.add)
            nv.scalar_tensor_tensor(hi_g, l_wh, 3.5, l_xy, OP.mult, OP.add)

            # ---- areas (unscaled): a = w*h ----
            _, aa, _ = t4("aa", 2, 49)   # a1, a2
            _, ag = t2("ag", 49)
            for x in range(2):
                nv.tensor_tensor(aa[:, :, x, :], PTg[:, :, x, 98:147],
                                 PTg[:, :, x, 147:196], OP.mult)
            nv.tensor_tensor(ag, l_w, l_h, OP.mult)
            _, ss, ss3 = t4("ss", 2, 49)   # a_k + ag
            nv.tensor_tensor(ss[:, :, 0, :], aa[:, :, 0, :], ag, OP.add)
            nv.tensor_tensor(ss[:, :, 1, :], aa[:, :, 1, :], ag, OP.add)

            # ---- intersection ----
            _, mx, mx3 = t4("mx", 2, 98)
            _, mn, mn3 = t4("mn", 2, 98)
            nv.tensor_tensor(mx[:, :, 0, :], lo_p[:, :, 0, :], lo_g, OP.max)
            nv.tensor_tensor(mx[:, :, 1, :], lo_p[:, :, 1, :], lo_g, OP.max)
            nv.tensor_tensor(mn[:, :, 0, :], hi_p[:, :, 0, :], hi_g, OP.min)
            nv.tensor_tensor(mn[:, :, 1, :], hi_p[:, :, 1, :], hi_g, OP.min)
            _, _, dd3 = t4("dd", 2, 98)
            nv.tensor_tensor(dd3, mn3, mx3, OP.subtract)
            na.activation(dd3, dd3, AF.Relu)
            _, _, ii3 = t4("ii", 2, 49)   # inter_raw (x49)
            nv.tensor_tensor(ii3, dd3[:, :, 0:49], dd3[:, :, 49:98], OP.mult)

            # ---- iou = inter_raw / (49*(a+ag) - inter_raw) ----
            _, _, dn3 = t4("dn", 2, 49)
            nv.scalar_tensor_tensor(dn3, ss3, 49.0, ii3, OP.mult, OP.subtract)
            _, _, rc3 = t4("rc", 2, 49)
            nv.reciprocal(rc3, dn3)
            _, io, io3 = t4("io", 2, 49)
            nv.tensor_tensor(io3, ii3, rc3, OP.mult)

            _, resp = t2("resp", 49, sml)
            nv.tensor_tensor(resp, io[:, :, 0, :], io[:, :, 1, :], OP.is_ge)

            # ---- conf terms: objc_k = (p_conf_k - iou_k)^2 ----
            _, dcp, dcp3 = t4("dcp", 2, 49)
            for x in range(2):
                nv.tensor_tensor(dcp[:, :, x, :], PTg[:, :, x, 196:245],
                                 io[:, :, x, :], OP.subtract)
            na.activation(dcp3, dcp3, AF.Square)   # -> objc1, objc2

            # ---- coor terms (x5 folded into squares) ----
            _, dxy, dxy3 = t4("dxy", 2, 98)
            nv.tensor_tensor(dxy[:, :, 0, :], PTg[:, :, 0, 0:98], l_xy,
                             OP.subtract)
            nv.tensor_tensor(dxy[:, :, 1, :], PTg[:, :, 1, 0:98], l_xy,
                             OP.subtract)
            na.activation(dxy3, dxy3, AF.Square, scale=SQ5)  # 5*(dxy)^2
            _, sp, _ = t4("sp", 2, 98)
            for x in range(2):
                na.activation(sp[:, :, x, :], PTg[:, :, x, 98:196], AF.Sqrt)
            _, sl = t2("sl", 98)
            na.activation(sl, l_wh, AF.Sqrt)
            _, ee, ee3 = t4("ee", 2, 98)
            nv.tensor_tensor(ee[:, :, 0, :], sp[:, :, 0, :], sl, OP.subtract)
            nv.tensor_tensor(ee[:, :, 1, :], sp[:, :, 1, :], sl, OP.subtract)
            na.activation(ee3, ee3, AF.Square, scale=SQ5)    # 5*(e)^2
            nv.tensor_tensor(dxy3, dxy3, ee3, OP.add)        # g (in-place)
            _, cc, cc3 = t4("cc", 2, 49)
            nv.tensor_tensor(cc3, dxy3[:, :, 0:49], dxy3[:, :, 49:98],
                             OP.add)                          # 5*coor1, 5*coor2

            # ---- cls = sum_c (p_c - l_c)^2 over 20 channels ----
            import os
            np_eng = ng if os.environ.get("KERNEL_POOL", "1") == "1" else nv
            _, dk = t2("dk", 980)
            np_eng.tensor_tensor(dk, p_cls, l_cls, OP.subtract)
            na.activation(dk, dk, AF.Square)
            _, u1 = t2("u1", 490)
            np_eng.tensor_tensor(u1, dk[:, :, 0:490], dk[:, :, 490:980],
                                 OP.add)
            _, u2 = t2("u2", 196)
            np_eng.tensor_tensor(u2, u1[:, :, 0:196], u1[:, :, 196:392],
                                 OP.add)
            _, u3 = t2("u3", 98, sml)
            nv.tensor_tensor(u3, u2[:, :, 0:98], u2[:, :, 98:196], OP.add)
            _, u4 = t2("u4", 49, sml)
            nv.tensor_tensor(u4, u3[:, :, 0:49], u3[:, :, 49:98], OP.add)
            _, u5 = t2("u5", 49, sml)
            nv.tensor_tensor(u5, u1[:, :, 392:441], u1[:, :, 441:490], OP.add)
            _, cls = t2("cls", 49, sml)
            nv.tensor_tensor(cls, u4, u5, OP.add)

            # ---- combine ----
            _, dobj = t2("dobj", 49, sml)
            nv.tensor_tensor(dobj, dcp[:, :, 0, :], dcp[:, :, 1, :],
                             OP.subtract)
            _, dcoor = t2("dcoor", 49, sml)
            nv.tensor_tensor(dcoor, cc[:, :, 0, :], cc[:, :, 1, :],
                             OP.subtract)
            _, du = t2("du", 49, sml)
            nv.scalar_tensor_tensor(du, dobj, 0.5, dcoor, OP.mult, OP.add)
            _, u2a = t2("u2a", 49, sml)
            nv.scalar_tensor_tensor(u2a, dcp[:, :, 0, :], 0.5,
                                    dcp[:, :, 1, :], OP.mult, OP.add)
            _, U2 = t2("U2", 49, sml)
            nv.tensor_tensor(U2, u2a, cc[:, :, 1, :], OP.add)
            _, sU = t2("sU", 49, sml)
            nv.tensor_tensor(sU, resp, du, OP.mult)
            _, selU = t2("selU", 49, sml)
            nv.tensor_tensor(selU, U2, sU, OP.add)
            _, inner = t2("inner", 49, sml)
            nv.tensor_tensor(inner, selU, cls, OP.add)

            _, hp, _ = t4("hp", 2, 49)
            for x in range(2):
                na.activation(hp[:, :, x, :], PTg[:, :, x, 196:245],
                              AF.Square, scale=ISQ2)  # 0.5*conf^2
            _, sph = t2("sph", 49, sml)
            nv.tensor_tensor(sph, hp[:, :, 0, :], hp[:, :, 1, :], OP.add)
            _, vv = t2("vv", 49, sml)
            nv.tensor_tensor(vv, inner, sph, OP.subtract)
            _, wv = t2("wv", 49, sml)
            nv.tensor_tensor(wv, l_obj, vv, OP.mult)

            cell_t, cell = t2("cell", 49, sml)
            nv.tensor_tensor(cell, wv, sph, OP.add)
            nv.tensor_reduce(acc[:, c:c + 1], cell_t[:],
                             mybir.AxisListType.X, OP.add)

        nc.sync.dma_start(out_ap, acc[:])


_NC_CACHE = None


def build_nc():
    global _NC_CACHE
    if _NC_CACHE is not None:
        return _NC_CACHE
    nc = bacc.Bacc(
        "TRN2",
        target_bir_lowering=False,
        debug=False,
        enable_asserts=False,
        num_devices=NCORES,
    )
    pred = nc.dram_tensor("pred", [BLOC, PREDW], F32, kind="ExternalInput")
    labs = nc.dram_tensor("labs", [BLOC, LABW], F32, kind="ExternalInput")
    out = nc.dram_tensor("out", [P, NCHUNK], F32, kind="ExternalOutput")
    with tile.TileContext(nc) as tc:
        _body(tc, pred.ap(), labs.ap(), out.ap())
    nc.compile()
    _NC_CACHE = nc
    return nc


def make_in_maps(pred, labels):
    pred = np.ascontiguousarray(np.asarray(pred, dtype=np.float32))
    labels = np.asarray(labels, dtype=np.float32)
    pred2 = pred.reshape(B, PREDW)
    lab2 = np.concatenate(
        [labels[:, 0:5], labels[:, 10:30]], axis=1
    ).reshape(B, LABW).astype(np.float32)
    return [
        {
            "pred": np.ascontiguousarray(pred2[i * BLOC:(i + 1) * BLOC]),
            "labs": np.ascontiguousarray(lab2[i * BLOC:(i + 1) * BLOC]),
        }
        for i in range(NCORES)
    ]


def run(pred, labels, trace=False, **kw):
    nc = build_nc()
    in_maps = make_in_maps(pred, labels)
    res = run_bass_kernel_spmd(
        nc, in_maps, core_ids=list(range(NCORES)), trace=trace, **kw)
    total = np.float64(0.0)
    for r in res.results:
        total += r["out"].astype(np.float64).sum()
    loss = np.float32(total / B)
    return loss, res


def kernel(pred, labels):
    loss, _ = run(pred, labels)
    return np.array(loss, dtype=np.float32)
